# revision 1
# baseline (speedup 1.0000x reference)
"""Trainium2 Bass kernel for AxialShiftedBlock (nn_AS_MLP): full-input API.

Sharding: data-parallel over batch B=16 -> 2 samples per core on 8 cores.
Per sample everything stays in SBUF: GN1 -> conv1x1 -> GN -> gelu -> axial
shifts -> conv1x1 x2 -> gelu -> add -> GN -> conv1x1 (+residual) -> GN ->
MLP (fc1 gelu fc2, +residual).

GroupNorm(1) applies that feed a linear conv are folded into that conv's
weights (W' = W * A per in-channel, b' = b + W @ B), recomputed on device
per sample.  Matmuls run in bf16 with fp32 PSUM accumulation; statistics
are computed in fp32 via bn_stats/bn_aggr (+ cross-partition reduction
with a ones-matmul on the tensor engine).
"""

import sys

sys.path.insert(0, "/opt/trn_rl_repo")

import numpy as np

# ---------------------------------------------------------------------------
# Workaround: this walrus build rejects >2 sync-wait slots on the CTRL/Drain
# instruction that TileContext emits at kernel exit ("Too many sync wait
# commands").  Split the global-clock waits across nop instructions emitted
# just before the drain (same engine, in-order => semantically identical).
# ---------------------------------------------------------------------------
import concourse.tile as _tile
import concourse.mybir as mybir
from concourse.vector_clock import ScopedClock as _ScopedClock


def _patched_drain_and_barrier(self, tick_clock, wait_clock):
    probe = self.nc.sync.nop(nofuse=True)
    wait_clock.add_sem_waits(probe.ins, _ScopedClock({None: tick_clock.global_clock}))
    waits = list(probe.ins.sync_info.on_wait)
    probe.ins.sync_info.on_wait = waits[:1]
    rest = waits[1:]
    for i in range(len(rest)):
        n = self.nc.sync.nop(nofuse=True)
        n.ins.sync_info = mybir.SyncInfo(on_wait=[rest[i]], on_update=[])
    self.nc.sync.drain()
    self.nc.all_engine_barrier()
    popped = self.nc._tile_sem_poison_stack.pop()
    assert popped is self._sem_poison
    self.nc.clear_and_free_semaphores(list(self.sems.allocated().values()))
    self.nc.all_engine_barrier()


_tile.TileContext._drain_and_barrier = _patched_drain_and_barrier

import concourse.bass as bass
from concourse.bass_utils import run_bass_kernel_spmd
from concourse.tile import TileContext
from concourse.masks import make_identity

F32 = mybir.dt.float32
BF16 = mybir.dt.bfloat16
ALU = mybir.AluOpType
ACTF = mybir.ActivationFunctionType

B, C, H, W = 16, 256, 56, 56
HW = H * W  # 3136
HID = 1024
NCORES = 8
BLOC = B // NCORES  # 2 samples per core
P = 128
KT = C // P  # 2 channel tiles
FT = HID // P  # 8 hidden tiles
NT = 7
NTILE = HW // NT  # 448
EPS = 1e-5

# axial shift: s = 3 - (c // 37); per 128-partition tile: (p0, p1, shift)
SHIFT_GROUPS = [
    [(0, 37, 3), (37, 74, 2), (74, 111, 1), (111, 128, 0)],
    [(0, 20, 0), (20, 57, -1), (57, 94, -2), (94, 128, -3)],
]

WEIGHT_NAMES = [
    "n1_w", "n1_b", "as_c1_w", "as_c1_b", "as_n1_w", "as_n1_b",
    "as_c21_w", "as_c21_b", "as_c22_w", "as_c22_b", "as_n2_w", "as_n2_b",
    "as_c3_w", "as_c3_b", "n2_w", "n2_b", "fc1_w", "fc1_b", "fc2_w", "fc2_b",
]


def _build_nc(dbg=None):
    nc = bass.Bass()

    x_d = nc.declare_dram_parameter("x", [BLOC, C, HW], F32, isOutput=False)
    y_d = nc.declare_dram_parameter("y", [BLOC, C, HW], F32, isOutput=True)
    wd = {}
    # weight matrices are pre-transposed to [cin, cout] (lhsT layout) on the
    # host; the static (non-GN-folded) ones additionally pre-cast to bf16
    shapes = {
        "n1_w": ([C], F32), "n1_b": ([C], F32),
        "as_c1_w": ([C, C], F32), "as_c1_b": ([C], F32),
        "as_n1_w": ([C], F32), "as_n1_b": ([C], F32),
        "as_c21_w": ([C, C], BF16), "as_c21_b": ([C], F32),
        "as_c22_w": ([C, C], BF16), "as_c22_b": ([C], F32),
        "as_n2_w": ([C], F32), "as_n2_b": ([C], F32),
        "as_c3_w": ([C, C], F32), "as_c3_b": ([C], F32),
        "n2_w": ([C], F32), "n2_b": ([C], F32),
        "fc1_w": ([C, HID], F32), "fc1_b": ([HID], F32),
        "fc2_w": ([HID, C], BF16), "fc2_b": ([C], F32),
    }
    for n in WEIGHT_NAMES:
        wd[n] = nc.declare_dram_parameter(n, shapes[n][0], shapes[n][1],
                                          isOutput=False)

    with TileContext(nc) as tc:
        _emit(nc, tc, x_d, y_d, wd, dbg)
    _split_sync_waits(nc)
    return nc


def _split_sync_waits(nc, max_waits=1):
    """Walrus in this container caps sync-wait slots per instruction; hoist
    excess waits onto same-engine nops inserted just before the instruction
    (same engine + in-order dispatch => semantically identical)."""
    nid = 0
    for fn in nc.m.functions:
        for blk in fn.blocks:
            out, changed = [], False
            for inst in list(blk.instructions):
                si = getattr(inst, "sync_info", None)
                if si is not None and len(si.on_wait) > max_waits:
                    waits = list(si.on_wait)
                    extra, keep = waits[:-max_waits], waits[-max_waits:]
                    for i0 in range(0, len(extra), max_waits):
                        nop = mybir.InstNoOp(name=f"I-wsplit-{nid}", ins=[], outs=[])
                        nid += 1
                        nop.engine = inst.engine
                        nop.sync_info = mybir.SyncInfo(
                            on_wait=extra[i0:i0 + max_waits], on_update=[]
                        )
                        out.append(nop)
                    si.on_wait = keep
                    changed = True
                out.append(inst)
            if changed:
                blk.instructions = out


PHASE_MARKS = []


def _emit(nc, tc, x_d, y_d, wd, dbg=None):
    from contextlib import ExitStack

    PHASE_MARKS.clear()

    def mark(label):
        PHASE_MARKS.append((label, nc.next_id()))

    with ExitStack() as ctx:
        consts = ctx.enter_context(tc.tile_pool(name="consts", bufs=1))
        wf32 = ctx.enter_context(tc.tile_pool(name="wf32", bufs=1))
        wbf = ctx.enter_context(tc.tile_pool(name="wbf", bufs=1))
        wsamp = ctx.enter_context(tc.tile_pool(name="wsamp", bufs=2))
        cols = ctx.enter_context(tc.tile_pool(name="cols", bufs=1))
        bigf = ctx.enter_context(tc.tile_pool(name="bigf", bufs=5))
        bigb = ctx.enter_context(tc.tile_pool(name="bigb", bufs=13))
        chk = ctx.enter_context(tc.tile_pool(name="chk", bufs=5))
        gchk = ctx.enter_context(tc.tile_pool(name="gchk", bufs=6))
        hchk = ctx.enter_context(tc.tile_pool(name="hchk", bufs=12))
        stat = ctx.enter_context(tc.tile_pool(name="stat", bufs=4))
        tiny = ctx.enter_context(tc.tile_pool(name="tiny", bufs=6))
        psmm = ctx.enter_context(tc.tile_pool(name="psmm", bufs=2, space="PSUM"))
        psml = ctx.enter_context(tc.tile_pool(name="psml", bufs=5, space="PSUM"))
        pssm = ctx.enter_context(tc.tile_pool(name="pssm", bufs=1, space="PSUM"))

        # ---------------- constants ----------------
        ones_col = consts.tile([P, 1], F32)
        nc.vector.memset(ones_col, 1.0)
        ones_row = consts.tile([1, P], F32)
        nc.vector.memset(ones_row, 1.0)
        eps_t = consts.tile([1, 1], F32)
        nc.vector.memset(eps_t, EPS)

        def dump(tiles, s):
            """Debug: write tiles (any dtype) to y for sample s."""
            for k, t in enumerate(tiles):
                if t.dtype == F32:
                    nc.sync.dma_start(out=y_d[s, k * P:(k + 1) * P, :], in_=t)
                else:
                    f = bigf.tile([P, HW], F32, tag="bigf", name="dbgf")
                    nc.scalar.copy(f, t)
                    nc.sync.dma_start(out=y_d[s, k * P:(k + 1) * P, :], in_=f)

        # ---------------- load vectors as [128,1] column tiles ------------
        def load_cols(name):
            n = wd[name].shape[0] // P
            v = wd[name].rearrange("(t p one) -> t p one", t=n, one=1)
            out = []
            for t in range(n):
                c = cols.tile([P, 1], F32, tag=f"col_{name}_{t}", name=f"col_{name}_{t}")
                nc.sync.dma_start(out=c, in_=v[t])
                out.append(c)
            return out

        n1w_c, n1b_c = load_cols("n1_w"), load_cols("n1_b")
        an1w_c, an1b_c = load_cols("as_n1_w"), load_cols("as_n1_b")
        an2w_c, an2b_c = load_cols("as_n2_w"), load_cols("as_n2_b")
        n2w_c, n2b_c = load_cols("n2_w"), load_cols("n2_b")
        b21_c, b22_c = load_cols("as_c21_b"), load_cols("as_c22_b")
        b1_c = load_cols("as_c1_b")
        b3_c = load_cols("as_c3_b")
        bfc1_c = load_cols("fc1_b")
        bfc2_c = load_cols("fc2_b")

        # --------- load weights (host pre-transposed to lhsT layout) -------
        def load_wT(name, cout, cin, tag, dtype=F32):
            """One DMA for the whole matrix into [128, kt*cout]; returns the
            per-k-tile column-slice views."""
            kt = cin // P
            pool = wf32 if dtype == F32 else wbf
            t = pool.tile([P, kt * cout], dtype, tag=tag, name=tag)
            # SWDGE keeps HWDGE free for latency-critical data DMAs
            nc.gpsimd.dma_start(
                out=t.rearrange("p (f c) -> p f c", f=kt),
                in_=wd[name].rearrange("(f p) c -> p f c", p=P),
            )
            return [t[:, k * cout:(k + 1) * cout] for k in range(kt)]

        # prefetch input samples first, in quarter-tiles so the first
        # bn_stats chunks unblock as soon as the leading bytes land
        x_pre = []
        for s_i in range(BLOC):
            per = []
            for k in range(KT):
                stg = bigf.tile([P, HW], F32, tag="bigf", name="x1f")
                for q in range(4):
                    qs = slice(q * (HW // 4), (q + 1) * (HW // 4))
                    nc.sync.dma_start(
                        out=stg[:, qs], in_=x_d[s_i, k * P:(k + 1) * P, qs]
                    )
                per.append(stg)
            x_pre.append(per)

        w1f = load_wT("as_c1_w", C, C, "w1f")
        w3f = load_wT("as_c3_w", C, C, "w3f")
        wfc1f = load_wT("fc1_w", HID, C, "wfc1f")
        # static bf16 weights (no GN fold): c21, c22, fc2
        w21b = load_wT("as_c21_w", C, C, "w21b", dtype=BF16)
        w22b = load_wT("as_c22_w", C, C, "w22b", dtype=BF16)
        wfc2b = load_wT("fc2_w", C, HID, "wfc2b", dtype=BF16)

        # ---------------- helpers ----------------
        def gn_tail(mv_tiles, n_ch, raw=False):
            """High priority: this chain of tiny ops gates a whole downstream
            phase, so it must cut ahead of bulk streaming work."""
            with tc.high_priority():
                return _gn_tail_body(mv_tiles, n_ch, raw)

        def _gn_tail_body(mv_tiles, n_ch, raw=False):
            if not raw:
                for mv in mv_tiles:
                    # mv := (mean, var + mean^2) in place
                    nc.vector.scalar_tensor_tensor(
                        out=mv[:, 1:2], in0=mv[:, 0:1], scalar=mv[:, 0:1],
                        in1=mv[:, 1:2], op0=ALU.mult, op1=ALU.add,
                    )
            pst = pssm.tile([1, 2], F32, tag="ps_small", name="ps_small")
            for i, u in enumerate(mv_tiles):
                nc.tensor.matmul(
                    pst, lhsT=ones_col, rhs=u,
                    start=(i == 0), stop=(i == len(mv_tiles) - 1),
                )
            inv = 1.0 / n_ch
            mr = tiny.tile([1, 2], F32, tag="gn_mr", name="gn_mr")
            mean = mr[:, 0:1]
            nc.vector.tensor_scalar_mul(mean, pst[0:1, 0:1], inv)
            ex2 = tiny.tile([1, 1], F32, tag="gn_ex2", name="gn_ex2")
            nc.vector.tensor_scalar_mul(ex2, pst[0:1, 1:2], inv)
            var = tiny.tile([1, 1], F32, tag="gn_var", name="gn_var")
            # var = -(mean*mean - ex2)
            nc.vector.scalar_tensor_tensor(
                out=var, in0=mean, scalar=mean, in1=ex2,
                op0=ALU.mult, op1=ALU.subtract,
            )
            nc.vector.tensor_scalar_mul(var, var, -1.0)
            sd = tiny.tile([1, 1], F32, tag="gn_sd", name="gn_sd")
            nc.scalar.activation(sd, var, ACTF.Sqrt, bias=eps_t, scale=1.0)
            nc.vector.reciprocal(mr[:, 1:2], sd)
            psb = pssm.tile([P, 2], F32, tag="ps_small", name="ps_small")
            nc.tensor.matmul(psb, lhsT=ones_row, rhs=mr)
            bc = tiny.tile([P, 2], F32, tag="gn_bc", name="gn_bc")
            nc.vector.tensor_copy(bc, psb)
            return bc[:, 0:1], bc[:, 1:2]

        def gn_ab(mean_b, rstd_b, w_cols, b_cols):
            """Per-channel A = rstd*w, B = b - mean*A, per tile."""
            with tc.high_priority():
                return _gn_ab_body(mean_b, rstd_b, w_cols, b_cols)

        def _gn_ab_body(mean_b, rstd_b, w_cols, b_cols):
            negm = tiny.tile([P, 1], F32, tag="gn_negm", name="gn_negm")
            nc.vector.tensor_scalar_mul(negm, mean_b, -1.0)
            A, Bc = [], []
            for t in range(len(w_cols)):
                a = tiny.tile([P, 1], F32, tag=f"gn_A{t}", name=f"gn_A{t}")
                nc.vector.tensor_mul(a, rstd_b, w_cols[t])
                b = tiny.tile([P, 1], F32, tag=f"gn_B{t}", name=f"gn_B{t}")
                nc.vector.scalar_tensor_tensor(
                    out=b, in0=a, scalar=negm, in1=b_cols[t],
                    op0=ALU.mult, op1=ALU.add,
                )
                A.append(a)
                Bc.append(b)
            return A, Bc

        def fold_weights(wf_tiles, A, Bc, b_cols, cout, tag, dtype=BF16):
            """W' = W * A (per in-channel); bias cols b' = b + (W @ B)."""
            with tc.high_priority():
                return _fold_weights_body(wf_tiles, A, Bc, b_cols, cout, tag,
                                          dtype)

        def _fold_weights_body(wf_tiles, A, Bc, b_cols, cout, tag, dtype):
            wp = []
            for k, wt in enumerate(wf_tiles):
                t = wsamp.tile([P, cout], dtype, tag=f"{tag}_w_{k}", name=f"{tag}_w_{k}")
                nc.vector.tensor_scalar_mul(t, wt, A[k])
                wp.append(t)
            bp = []
            for f in range(cout // P):
                psr = pssm.tile([P, 1], F32, tag="ps_small", name="ps_small")
                for k, wt in enumerate(wf_tiles):
                    nc.tensor.matmul(
                        psr, lhsT=wt[:, f * P:(f + 1) * P], rhs=Bc[k],
                        start=(k == 0), stop=(k == len(wf_tiles) - 1),
                    )
                bc = wsamp.tile([P, 1], F32, tag=f"{tag}_bc{f}", name=f"{tag}_bc{f}")
                nc.vector.tensor_add(bc, psr, b_cols[f])
                bp.append(bc)
            return wp, bp

        # ============ phase-interleaved two-sample pipeline ============
        # Emitting matched phases of the two samples adjacently gives the
        # greedy scheduler alternating priorities, so sample 1's front
        # overlaps sample 0's MLP instead of queueing behind it.
        def ph_stats(S):
            s = S["s"]
            mark(f's{s}_load')
            x1f, xb, mv1 = x_pre[s], [], []
            for k in range(KT):
                t = bigb.tile([P, HW], BF16, tag="bigb", name="xb")
                nc.gpsimd.tensor_copy(out=t, in_=x1f[k])
                xb.append(t)
                st = stat.tile([P, NT, 6], F32, tag="st_gn1", name="st_gn1")
                for j in range(NT):
                    nc.vector.bn_stats(
                        st[:, j, :], x1f[k][:, j * NTILE:(j + 1) * NTILE]
                    )
                mv = stat.tile([P, 2], F32, tag="mv_gn1", name="mv_gn1")
                nc.vector.bn_aggr(mv, st)
                mv1.append(mv)
            S.update(x1f=x1f, xb=xb, mv1=mv1)

        def ph_gn1tail(S):
            s = S["s"]
            mark(f's{s}_gn1tail')
            mean1, rstd1 = gn_tail(S["mv1"], C)
            A1, B1 = gn_ab(mean1, rstd1, n1w_c, n1b_c)
            w1p, b1p = fold_weights(w1f, A1, B1, b1_c, C, "w1p")
            S.update(w1p=w1p, b1p=b1p)

        def ph_c1(S):
            s, x1f, xb = S["s"], S["x1f"], S["xb"]
            w1p, b1p = S["w1p"], S["b1p"]
            mark(f's{s}_c1')
            zb = [bigb.tile([P, HW], BF16, tag="bigb", name="zb") for _ in range(KT)]
            zst = [stat.tile([P, NT, 6], F32, tag="st_z", name="st_z") for _ in range(KT)]
            for j in range(NT):
                js = slice(j * NTILE, (j + 1) * NTILE)
                for m in range(KT):
                    ps = psmm.tile([P, NTILE], F32, tag="ps_mm", name="ps_mm")
                    for k in range(KT):
                        nc.tensor.matmul(
                            ps, lhsT=w1p[k][:, m * P:(m + 1) * P], rhs=xb[k][:, js],
                            start=(k == 0), stop=(k == KT - 1),
                        )
                    nc.scalar.activation(zb[m][:, js], ps, ACTF.Identity, bias=b1p[m])
                    nc.vector.bn_stats(zst[m][:, j, :], zb[m][:, js])
            S.update(zb=zb, zst=zst)

        def ph_ztail(S):
            s = S["s"]
            mark(f's{s}_zttail')
            mvz = []
            for m in range(KT):
                mv = stat.tile([P, 2], F32, tag="mv_z", name="mv_z")
                nc.vector.bn_aggr(mv, S["zst"][m])
                mvz.append(mv)
            mean2, rstd2 = gn_tail(mvz, C)
            A2, B2 = gn_ab(mean2, rstd2, an1w_c, an1b_c)
            S.update(A2=A2, B2=B2)

        def ph_apply(S):
            s, zb, A2, B2 = S["s"], S["zb"], S["A2"], S["B2"]
            mark(f's{s}_apply')
            tb = zb
            for m in range(KT):
                for j2 in range(2):
                    cs = slice(j2 * 1568, (j2 + 1) * 1568)
                    nc.scalar.activation(
                        tb[m][:, cs], zb[m][:, cs], ACTF.Gelu,
                        bias=B2[m], scale=A2[m],
                    )
            S["tb"] = tb

        def ph_shift(S):
            s, tb = S["s"], S["tb"]
            mark(f's{s}_shift')
            tlr = [bigb.tile([P, HW], BF16, tag="bigb", name="tlr") for _ in range(KT)]
            ttd = [bigb.tile([P, HW], BF16, tag="bigb", name="ttd") for _ in range(KT)]
            PADS = 3
            for k in range(KT):
                t3 = tlr[k].rearrange("p (h w) -> p h w", w=W)
                s3 = tb[k].rearrange("p (h w) -> p h w", w=W)
                nc.vector.memset(t3[:, :, 0:PADS], 0.0)
                nc.vector.memset(t3[:, :, W - PADS:W], 0.0)
                for (p0, p1, sv) in SHIFT_GROUPS[k]:
                    if sv == 0:
                        nc.sync.dma_start(
                            out=tlr[k][p0:p1, :], in_=tb[k][p0:p1, :]
                        )
                    elif sv > 0:
                        nc.sync.dma_start(
                            out=t3[p0:p1, :, 0:W - sv], in_=s3[p0:p1, :, sv:W]
                        )
                    else:
                        a = -sv
                        nc.sync.dma_start(
                            out=t3[p0:p1, :, a:W], in_=s3[p0:p1, :, 0:W - a]
                        )
                nc.vector.memset(ttd[k][:, 0:PADS * W], 0.0)
                nc.vector.memset(ttd[k][:, HW - PADS * W:HW], 0.0)
                for (p0, p1, sv) in SHIFT_GROUPS[k]:
                    # out[pos] = in[pos + a] for pos with 0 <= pos+a < HW
                    a = sv * W
                    c0, c1 = max(0, -a), min(HW, HW - a)
                    nc.gpsimd.dma_start(
                        out=ttd[k][p0:p1, c0:c1],
                        in_=tb[k][p0:p1, c0 + a:c1 + a],
                    )
            S.update(tlr=tlr, ttd=ttd)

        def ph_c21(S):
            s, tlr, ttd = S["s"], S["tlr"], S["ttd"]
            mark(f's{s}_c21')
            sb = [bigb.tile([P, HW], BF16, tag="bigb", name="sb") for _ in range(KT)]
            sst = [stat.tile([P, NT, 6], F32, tag="st_s", name="st_s") for _ in range(KT)]
            for j in range(NT):
                js = slice(j * NTILE, (j + 1) * NTILE)
                for m in range(KT):
                    ps1 = psmm.tile([P, NTILE], F32, tag="ps_mm", name="ps_mm")
                    for k in range(KT):
                        nc.tensor.matmul(
                            ps1, lhsT=w21b[k][:, m * P:(m + 1) * P],
                            rhs=tlr[k][:, js], start=(k == 0), stop=(k == KT - 1),
                        )
                    g1 = gchk.tile([P, NTILE], BF16, tag="gchk", name="g1")
                    nc.scalar.activation(g1, ps1, ACTF.Gelu, bias=b21_c[m])
                    ps2 = psmm.tile([P, NTILE], F32, tag="ps_mm", name="ps_mm")
                    for k in range(KT):
                        nc.tensor.matmul(
                            ps2, lhsT=w22b[k][:, m * P:(m + 1) * P],
                            rhs=ttd[k][:, js], start=(k == 0), stop=(k == KT - 1),
                        )
                    g2 = gchk.tile([P, NTILE], BF16, tag="gchk", name="g2")
                    nc.scalar.activation(g2, ps2, ACTF.Gelu, bias=b22_c[m])
                    nc.vector.tensor_add(sb[m][:, js], g1, g2)
                    nc.vector.bn_stats(sst[m][:, j, :], sb[m][:, js])
            S.update(sb=sb, sst=sst)

        def ph_stail(S):
            s = S["s"]
            mark(f's{s}_stail')
            mvs = []
            for m in range(KT):
                mv = stat.tile([P, 2], F32, tag="mv_s", name="mv_s")
                nc.vector.bn_aggr(mv, S["sst"][m])
                mvs.append(mv)
            mean3, rstd3 = gn_tail(mvs, C)
            A3, B3 = gn_ab(mean3, rstd3, an2w_c, an2b_c)
            w3p, b3p = fold_weights(w3f, A3, B3, b3_c, C, "w3p")
            S.update(w3p=w3p, b3p=b3p)

        def ph_c3(S):
            s, sb, x1f = S["s"], S["sb"], S["x1f"]
            w3p, b3p = S["w3p"], S["b3p"]
            mark(f's{s}_c3')
            ac1 = [stat.tile([P, NT], F32, tag="ac1", name="ac1") for _ in range(KT)]
            ac2 = [stat.tile([P, NT], F32, tag="ac2", name="ac2") for _ in range(KT)]
            for j in range(NT):
                js = slice(j * NTILE, (j + 1) * NTILE)
                for m in range(KT):
                    ps = psmm.tile([P, NTILE], F32, tag="ps_mm", name="ps_mm")
                    for k in range(KT):
                        nc.tensor.matmul(
                            ps, lhsT=w3p[k][:, m * P:(m + 1) * P],
                            rhs=sb[k][:, js], start=(k == 0), stop=(k == KT - 1),
                        )
                    nc.vector.scalar_tensor_tensor(
                        out=x1f[m][:, js], in0=ps, scalar=b3p[m],
                        in1=x1f[m][:, js], op0=ALU.add, op1=ALU.add,
                        accum_out=ac1[m][:, j:j + 1],
                    )
                    sq = gchk.tile([P, NTILE], BF16, tag="gchk", name="sqt")
                    nc.scalar.activation(sq, x1f[m][:, js], ACTF.Square,
                                         accum_out=ac2[m][:, j:j + 1])
            S.update(ac1=ac1, ac2=ac2)

        def ph_x1tail(S):
            s = S["s"]
            mark(f's{s}_x1tail')
            mvx1 = []
            for m in range(KT):
                u = stat.tile([P, 2], F32, tag="mv_x1", name="mv_x1")
                nc.vector.tensor_reduce(u[:, 0:1], S["ac1"][m], axis=mybir.AxisListType.X,
                                        op=ALU.add)
                nc.vector.tensor_reduce(u[:, 1:2], S["ac2"][m], axis=mybir.AxisListType.X,
                                        op=ALU.add)
                mvx1.append(u)
            mean4, rstd4 = gn_tail(mvx1, C * HW, raw=True)
            A4, B4 = gn_ab(mean4, rstd4, n2w_c, n2b_c)
            wfc1p, bfc1p = fold_weights(wfc1f, A4, B4, bfc1_c, HID, "wfc1p")
            S.update(wfc1p=wfc1p, bfc1p=bfc1p)

        def ph_mlp(S):
            s, x1f = S["s"], S["x1f"]
            wfc1p, bfc1p = S["wfc1p"], S["bfc1p"]
            mark(f's{s}_mlp')
            for j in range(NT):
                js = slice(j * NTILE, (j + 1) * NTILE)
                x1bc = []
                for k in range(KT):
                    cb = chk.tile([P, NTILE], BF16, tag="chk", name="x1bc")
                    nc.gpsimd.tensor_copy(out=cb, in_=x1f[k][:, js])
                    x1bc.append(cb)
                hj = []
                for f in range(FT):
                    ps = psml.tile([P, NTILE], F32, tag="ps_ml", name="ps_ml")
                    for k in range(KT):
                        nc.tensor.matmul(
                            ps, lhsT=wfc1p[k][:, f * P:(f + 1) * P], rhs=x1bc[k],
                            start=(k == 0), stop=(k == KT - 1),
                        )
                    hc = hchk.tile([P, NTILE], BF16, tag="hj", name="hj")
                    nc.scalar.activation(hc, ps, ACTF.Gelu, bias=bfc1p[f])
                    hj.append(hc)
                for m in range(KT):
                    ps = psml.tile([P, NTILE], F32, tag="ps_ml", name="ps_ml")
                    for f in range(FT):
                        nc.tensor.matmul(
                            ps, lhsT=wfc2b[f][:, m * P:(m + 1) * P], rhs=hj[f],
                            start=(f == 0), stop=(f == FT - 1),
                        )
                    nc.vector.scalar_tensor_tensor(
                        out=x1f[m][:, js], in0=ps, scalar=bfc2_c[m],
                        in1=x1f[m][:, js], op0=ALU.add, op1=ALU.add,
                    )
                    nc.sync.dma_start(
                        out=y_d[s, m * P:(m + 1) * P, j * NTILE:(j + 1) * NTILE],
                        in_=x1f[m][:, js],
                    )

        dump_after = {
            "x": (ph_stats, "xb"), "z": (ph_c1, "zb"), "t": (ph_apply, "tb"),
            "tlr": (ph_shift, "tlr"), "ttd": (ph_shift, "ttd"),
            "s": (ph_c21, "sb"), "x1": (ph_c3, "x1f"),
        }
        phases = [ph_stats, ph_gn1tail, ph_c1, ph_ztail, ph_apply, ph_shift,
                  ph_c21, ph_stail, ph_c3, ph_x1tail, ph_mlp]
        states = [{"s": si} for si in range(BLOC)]
        if dbg is not None:
            for ph in phases:
                for S in states:
                    ph(S)
                if dbg in dump_after and dump_after[dbg][0] is ph:
                    key = dump_after[dbg][1]
                    for S in states:
                        dump(S[key], S["s"])
                    return
            return
        for ph in phases:
            for S in states:
                ph(S)


_NC_CACHE = {}


def _get_nc():
    if "nc" not in _NC_CACHE:
        _NC_CACHE["nc"] = _build_nc()
    return _NC_CACHE["nc"]


def _run(inputs, **kw):
    import ml_dtypes

    x = np.ascontiguousarray(np.asarray(inputs["x"], np.float32)).reshape(B, C, HW)
    common = {}
    for n in WEIGHT_NAMES:
        a = np.asarray(inputs[n], np.float32)
        if a.ndim == 2:
            a = a.T
            if n in ("as_c21_w", "as_c22_w", "fc2_w"):
                a = a.astype(ml_dtypes.bfloat16)
        common[n] = np.ascontiguousarray(a)
    in_maps = [
        {**common, "x": np.ascontiguousarray(x[c * BLOC:(c + 1) * BLOC])}
        for c in range(NCORES)
    ]
    res = run_bass_kernel_spmd(_get_nc(), in_maps, list(range(NCORES)), **kw)
    y = np.concatenate([res.results[c]["y"] for c in range(NCORES)], axis=0)
    return y.reshape(B, C, H, W), res


def kernel(**inputs) -> np.ndarray:
    y, _ = _run(inputs)
    return y


def kernel_timed(**inputs):
    y, res = _run(inputs, trace=True)
    return y, res

